# revision 3
# baseline (speedup 1.0000x reference)
"""GCN encoder (3-layer, mu/logstd heads) on 8 Trainium2 NeuronCores.

Strategy:
- Nodes block-sharded across 8 cores (12500/core, padded to 12544 = 128*98).
- Per core, local dst nodes are sorted by in-degree; groups of 128 ranks form
  an ELL structure: group g, source-window r has width W[g][r] (cross-core max
  so the SPMD program is shape-uniform). Messages are fetched with
  dma_gather (int16 indices, so the 100352-row gathered table is split into
  4 windows of 25088 rows) and segment-summed with static strided DVE reduces.
- Normalization folded: table rows are pre-scaled by dinv[src]; aggregation
  output is scaled by dinv[dst]. Self-loops are ordinary edges.
- Layer outputs are AllGathered (8 ranks) into the next layer's full table.
- mu and logstd share one aggregation of x2 (linearity of A @ (x @ W)).
"""

import numpy as np

N = 100000
NC = 8
NLOC = N // NC            # 12500
G = 98                    # groups of 128 ranks
NPAD = 128 * G            # 12544
NFULL = NC * NPAD         # 100352
WIN = NFULL // 4          # 25088 rows per int16 gather window
F1, F2, F3, F4 = 128, 64, 32, 16
MAX_COLS_PER_GATHER = 64  # 64*128 = 8192 idxs max per dma_gather

_PROFILE = False
_last_exec_ns = None
_TMPDIR = None


def _wrap_idxs(idxs):
    """[n] -> [128, n/16] int16: slot j at [j%16, j//16], replicated x8."""
    n = len(idxs)
    assert n % 16 == 0
    w = idxs.reshape(n // 16, 16).T.astype(np.int16)
    return np.tile(w, (8, 1))


def _preprocess(edge_index):
    """Host-side: sharding, degree sort, ELL slot layout, gather indices."""
    src = np.asarray(edge_index[0], dtype=np.int64)
    dst = np.asarray(edge_index[1], dtype=np.int64)
    loop = np.arange(N, dtype=np.int64)
    src = np.concatenate([src, loop])
    dst = np.concatenate([dst, loop])

    deg = np.bincount(dst, minlength=N).astype(np.float64)
    dinv = np.where(deg > 0, 1.0 / np.sqrt(deg), 0.0).astype(np.float32)

    cores = []
    for c in range(NC):
        lo, hi = c * NLOC, (c + 1) * NLOC
        m = (dst >= lo) & (dst < hi)
        s_c = src[m]
        d_c = dst[m] - lo
        degloc = np.bincount(d_c, minlength=NLOC)
        order = np.argsort(-degloc, kind="stable")          # rank -> local node
        rank_of = np.empty(NLOC, np.int64)
        rank_of[order] = np.arange(NLOC)
        cores.append(dict(s=s_c, d=d_c, order=order, rank_of=rank_of))

    # global table row of a node: shard base + p*G + g where rank = 128*g + p
    row_of_node = np.empty(N, np.int64)
    for c in range(NC):
        rk = cores[c]["rank_of"]
        row_of_node[c * NLOC:(c + 1) * NLOC] = \
            c * NPAD + (rk % 128) * G + (rk // 128)

    for c in range(NC):
        cc = cores[c]
        trow = row_of_node[cc["s"]]
        cc["win"] = trow // WIN
        cc["lidx"] = trow % WIN
        cc["rank"] = cc["rank_of"][cc["d"]]

    # widths W[g][r]: max over cores and over the 128 group dsts
    cnt = np.zeros((NC, NPAD, 4), np.int32)
    for c in range(NC):
        cc = cores[c]
        np.add.at(cnt[c], (cc["rank"], cc["win"]), 1)
    W = cnt.reshape(NC, G, 128, 4).max(axis=2).max(axis=0)   # [G, 4]

    # guaranteed-zero row (local coords valid in every window)
    pad_rank = NLOC
    zero_local = (pad_rank % 128) * G + (pad_rank // 128)

    seg_meta = [(g, r, int(W[g][r]))
                for g in range(G) for r in range(4) if W[g][r] > 0]

    idx_streams = []
    for c in range(NC):
        cc = cores[c]
        ordk = np.lexsort((cc["lidx"], cc["win"], cc["rank"]))
        rk_s = cc["rank"][ordk]
        wn_s = cc["win"][ordk]
        li_s = cc["lidx"][ordk]
        key = rk_s * 4 + wn_s
        start = np.searchsorted(key, np.arange(NPAD * 4))
        end = np.searchsorted(key, np.arange(NPAD * 4) + 1)

        streams = {r: [] for r in range(4)}
        for (g, r, w) in seg_meta:
            seg = np.full((128, w), zero_local, np.int64)
            for p in range(128):
                rk = 128 * g + p
                a, b = start[rk * 4 + r], end[rk * 4 + r]
                if b > a:
                    seg[p, :b - a] = li_s[a:b]
            flat = seg.T.reshape(-1)           # slot j = 128*i + p
            streams[r].append(_wrap_idxs(flat))
        idx_streams.append({r: (np.concatenate(streams[r], axis=1)
                                if streams[r] else
                                np.zeros((128, 16), np.int16))
                            for r in range(4)})

    return dinv, cores, W, seg_meta, idx_streams


def _build_program(seg_meta, idx_len):
    import contextlib
    import concourse.bacc as bacc
    import concourse.mybir as mybir
    import concourse.tile as tile
    from concourse import library_config
    from concourse.masks import make_identity

    dt = mybir.dt
    Alu = mybir.AluOpType
    nc = bacc.Bacc("TRN2", target_bir_lowering=False, debug=False,
                   num_devices=NC)

    xT = nc.dram_tensor("xT", [128, NPAD], dt.float32, kind="ExternalInput")
    dinv_d = nc.dram_tensor("dinv", [128, G], dt.float32, kind="ExternalInput")
    w1_d = nc.dram_tensor("w1", [F1, F2], dt.float32, kind="ExternalInput")
    w2_d = nc.dram_tensor("w2", [F2, F3], dt.float32, kind="ExternalInput")
    wmu_d = nc.dram_tensor("wmu", [F3, F4], dt.float32, kind="ExternalInput")
    wls_d = nc.dram_tensor("wls", [F3, F4], dt.float32, kind="ExternalInput")
    b1_d = nc.dram_tensor("b1t", [128, F2], dt.float32, kind="ExternalInput")
    b2_d = nc.dram_tensor("b2t", [128, F3], dt.float32, kind="ExternalInput")
    bmu_d = nc.dram_tensor("bmut", [128, F4], dt.float32, kind="ExternalInput")
    bls_d = nc.dram_tensor("blst", [128, F4], dt.float32, kind="ExternalInput")
    idx_d = [nc.dram_tensor(f"idx{r}", [128, idx_len[r]], dt.int16,
                            kind="ExternalInput") for r in range(4)]
    mu_out = nc.dram_tensor("mu", [128, G, F4], dt.float32,
                            kind="ExternalOutput")
    ls_out = nc.dram_tensor("ls", [128, G, F4], dt.float32,
                            kind="ExternalOutput")

    FW = F2   # gather-table width: 64 f32 cols = 256B rows

    with tile.TileContext(nc) as tc:
        with contextlib.ExitStack() as ctx:
            dram = ctx.enter_context(
                tc.tile_pool(name="dram", bufs=1, space="DRAM"))
            consts = ctx.enter_context(tc.tile_pool(name="consts", bufs=1))
            psum_mm = ctx.enter_context(
                tc.tile_pool(name="psum_mm", bufs=3, space="PSUM"))
            psum_tr = ctx.enter_context(
                tc.tile_pool(name="psum_tr", bufs=2, space="PSUM"))
            tabp = ctx.enter_context(tc.tile_pool(name="tabp", bufs=1))
            aggp = ctx.enter_context(tc.tile_pool(name="aggp", bufs=1))
            smallp = ctx.enter_context(tc.tile_pool(name="smallp", bufs=4))

            nc.gpsimd.load_library(library_config.mlp)

            def cload(name, dram_t, shape):
                t = consts.tile(shape, dt.float32, name=name)
                nc.sync.dma_start(t[:], dram_t[:])
                return t

            dinv_sb = cload("dinv_sb", dinv_d, [128, G])
            w1_sb = cload("w1_sb", w1_d, [F1, F2])
            w2_sb = cload("w2_sb", w2_d, [F2, F3])
            wmu_sb = cload("wmu_sb", wmu_d, [F3, F4])
            wls_sb = cload("wls_sb", wls_d, [F3, F4])
            b1_sb = cload("b1_sb", b1_d, [128, F2])
            b2_sb = cload("b2_sb", b2_d, [128, F3])
            bmu_sb = cload("bmu_sb", bmu_d, [128, F4])
            bls_sb = cload("bls_sb", bls_d, [128, F4])
            ident = consts.tile([128, 128], dt.float32, name="ident")
            make_identity(nc, ident[:])

            def all_gather(loc, name):
                full = dram.tile([NFULL, FW], dt.float32,
                                 addr_space="Shared", name=name)
                nc.gpsimd.collective_compute(
                    "AllGather", Alu.bypass,
                    replica_groups=[list(range(NC))],
                    ins=[loc.opt()], outs=[full.opt()],
                )
                return full

            def store_table(tab_sb, name):
                loc = dram.tile([NPAD, FW], dt.float32, name=name)
                nc.sync.dma_start(
                    loc[:].rearrange("(p g) f -> p g f", p=128), tab_sb[:])
                return all_gather(loc, name + "_full")

            # ---------- Layer 1 matmul: tab1 = dinv * (x @ W1) ----------
            with tc.tile_pool(name="xTp", bufs=1) as xp:
                xT_sb = xp.tile([128, NPAD], dt.float32, name="xT_sb")
                nc.sync.dma_start(xT_sb[:], xT[:])
                tab_sb = tabp.tile([128, G, FW], dt.float32, tag="tab",
                                   name="tab1_sb")
                for g in range(G):
                    ps = psum_mm.tile([128, FW], dt.float32, space="PSUM",
                                      tag="mm", name=f"mm1_{g}")
                    nc.tensor.matmul(out=ps[:],
                                     lhsT=xT_sb[:, 128 * g:128 * (g + 1)],
                                     rhs=w1_sb[:], start=True, stop=True)
                    nc.vector.tensor_scalar_mul(
                        tab_sb[:, g, :], ps[:], dinv_sb[:, g:g + 1])
                tab1_full = store_table(tab_sb, "tab1")

            # idx streams loaded after xT is freed (SBUF budget)
            idxp = ctx.enter_context(tc.tile_pool(name="idxp", bufs=1))
            msgp = ctx.enter_context(tc.tile_pool(name="msgp", bufs=2))
            idx_sb = {}
            for r in range(4):
                t = idxp.tile([128, idx_len[r]], dt.int16, tag=f"idx{r}",
                              name=f"idxt{r}")
                nc.sync.dma_start(t[:], idx_d[r][:])
                idx_sb[r] = t

            seg_by_g = {}
            for (g, r, w) in seg_meta:
                seg_by_g.setdefault(g, []).append((r, w))

            def aggregate(tab_full, out_cb, phase):
                off = {0: 0, 1: 0, 2: 0, 3: 0}
                for g in range(G):
                    segs = seg_by_g[g]
                    wtot = sum(w for (_, w) in segs)
                    mt = msgp.tile([128, wtot, FW], dt.float32, tag="msg",
                                   name=f"msg_{phase}_{g}")
                    col = 0
                    for (r, w) in segs:
                        c0 = 0
                        while c0 < w:
                            cw = min(w - c0, MAX_COLS_PER_GATHER)
                            nc.gpsimd.dma_gather(
                                mt[:, col + c0:col + c0 + cw, :],
                                tab_full[r * WIN:(r + 1) * WIN, :],
                                idx_sb[r][:, off[r] + 8 * c0:
                                          off[r] + 8 * (c0 + cw)],
                                128 * cw, 128 * cw, FW,
                                single_packet=False,
                            )
                            c0 += cw
                        off[r] += 8 * w
                        col += w
                    red = smallp.tile([128, FW], dt.float32, tag="red",
                                      name=f"red_{phase}_{g}")
                    nc.vector.tensor_reduce(
                        red[:], mt[:].rearrange("p w f -> p f w"),
                        axis=mybir.AxisListType.X, op=Alu.add)
                    out_cb(g, red)

            # ---------- Layer 1 aggregate -> x1 ----------
            x1_sb = aggp.tile([128, G, F2], dt.float32, tag="x1",
                              name="x1_sb")

            def l1_post(g, red):
                nc.vector.tensor_scalar_mul(red[:], red[:],
                                            dinv_sb[:, g:g + 1])
                nc.vector.tensor_tensor(red[:], red[:], b1_sb[:], op=Alu.add)
                nc.vector.tensor_scalar(x1_sb[:, g, :], red[:], 0.0, None,
                                        Alu.max)

            aggregate(tab1_full, l1_post, "l1")

            # ---------- Layer 2: tab2 = dinv * (x1 @ W2), padded ----------
            tab_sb2 = tabp.tile([128, G, FW], dt.float32, tag="tab",
                                name="tab2_sb")
            nc.vector.memset(tab_sb2[:], 0.0)
            for g in range(G):
                pt = psum_tr.tile([F2, 128], dt.float32, space="PSUM",
                                  tag="tr", name=f"tr2_{g}")
                nc.tensor.transpose(pt[:], x1_sb[:, g, :], ident[:])
                x1t = smallp.tile([F2, 128], dt.float32, tag="x1t",
                                  name=f"x1t_{g}")
                nc.vector.tensor_copy(x1t[:], pt[:])
                ps = psum_mm.tile([128, FW], dt.float32, space="PSUM",
                                  tag="mm", name=f"mm2_{g}")
                nc.tensor.matmul(out=ps[:, 0:F3], lhsT=x1t[:], rhs=w2_sb[:],
                                 start=True, stop=True)
                nc.vector.tensor_scalar_mul(
                    tab_sb2[:, g, 0:F3], ps[:, 0:F3], dinv_sb[:, g:g + 1])
            tab2_full = store_table(tab_sb2, "tab2")

            x2_sb = aggp.tile([128, G, F3], dt.float32, tag="x2",
                              name="x2_sb")

            def l2_post(g, red):
                nc.vector.tensor_scalar_mul(red[:, 0:F3], red[:, 0:F3],
                                            dinv_sb[:, g:g + 1])
                nc.vector.tensor_tensor(red[:, 0:F3], red[:, 0:F3], b2_sb[:],
                                        op=Alu.add)
                nc.vector.tensor_scalar(x2_sb[:, g, :], red[:, 0:F3], 0.0,
                                        None, Alu.max)

            aggregate(tab2_full, l2_post, "l2")

            # ---------- Layer 3: tab3 = dinv * x2 ----------
            tab_sb3 = tabp.tile([128, G, FW], dt.float32, tag="tab",
                                name="tab3_sb")
            nc.vector.memset(tab_sb3[:], 0.0)
            for g in range(G):
                nc.vector.tensor_scalar_mul(
                    tab_sb3[:, g, 0:F3], x2_sb[:, g, :], dinv_sb[:, g:g + 1])
            tab3_full = store_table(tab_sb3, "tab3")

            mu_sb = aggp.tile([128, G, F4], dt.float32, tag="mu",
                              name="mu_sb")
            ls_sb = aggp.tile([128, G, F4], dt.float32, tag="lsb",
                              name="ls_sb")

            def l3_post(g, red):
                nc.vector.tensor_scalar_mul(red[:, 0:F3], red[:, 0:F3],
                                            dinv_sb[:, g:g + 1])
                pt = psum_tr.tile([F3, 128], dt.float32, space="PSUM",
                                  tag="tr", name=f"tr3_{g}")
                nc.tensor.transpose(pt[:], red[:, 0:F3], ident[:])
                zt = smallp.tile([F3, 128], dt.float32, tag="x1t",
                                 name=f"zt_{g}")
                nc.vector.tensor_copy(zt[:], pt[:])
                pmu = psum_mm.tile([128, FW], dt.float32, space="PSUM",
                                   tag="mm", name=f"pmu_{g}")
                nc.tensor.matmul(out=pmu[:, 0:F4], lhsT=zt[:], rhs=wmu_sb[:],
                                 start=True, stop=True)
                nc.vector.tensor_tensor(mu_sb[:, g, :], pmu[:, 0:F4],
                                        bmu_sb[:], op=Alu.add)
                pls = psum_mm.tile([128, FW], dt.float32, space="PSUM",
                                   tag="mm", name=f"pls_{g}")
                nc.tensor.matmul(out=pls[:, 0:F4], lhsT=zt[:], rhs=wls_sb[:],
                                 start=True, stop=True)
                nc.vector.tensor_tensor(ls_sb[:, g, :], pls[:, 0:F4],
                                        bls_sb[:], op=Alu.add)

            aggregate(tab3_full, l3_post, "l3")

            nc.sync.dma_start(mu_out[:], mu_sb[:])
            nc.sync.dma_start(ls_out[:], ls_sb[:])

    nc.compile()
    return nc


def kernel(x, edge_index, W1, b1, W2, b2, Wmu, bmu, Wls, bls):
    global _last_exec_ns
    x = np.asarray(x, np.float32)
    dinv, cores, W, seg_meta, idx_streams = _preprocess(edge_index)
    idx_len = [idx_streams[0][r].shape[1] for r in range(4)]

    nc = _build_program(seg_meta, idx_len)

    def btile(b):
        return np.tile(np.asarray(b, np.float32)[None, :], (128, 1))

    in_maps = []
    for c in range(NC):
        cc = cores[c]
        xT = np.zeros((128, NPAD), np.float32)
        # column rank holds x[node_of_rank]
        xT[:, cc["rank_of"]] = x[c * NLOC:(c + 1) * NLOC].T

        dv = np.zeros((128, G), np.float32)
        rr = np.arange(128)[:, None] + 128 * np.arange(G)[None, :]
        mreal = rr < NLOC
        dv[mreal] = dinv[c * NLOC + cc["order"][rr[mreal]]]

        im = dict(xT=xT, dinv=dv, w1=np.asarray(W1, np.float32),
                  w2=np.asarray(W2, np.float32),
                  wmu=np.asarray(Wmu, np.float32),
                  wls=np.asarray(Wls, np.float32),
                  b1t=btile(b1), b2t=btile(b2), bmut=btile(bmu),
                  blst=btile(bls))
        for r in range(4):
            im[f"idx{r}"] = idx_streams[c][r]
        in_maps.append(im)

    from concourse.bass_utils import run_bass_kernel_spmd
    res = run_bass_kernel_spmd(nc, in_maps, core_ids=list(range(NC)),
                               trace=_PROFILE, tmpdir=_TMPDIR)
    _last_exec_ns = res.exec_time_ns

    mu = np.empty((N, F4), np.float32)
    ls = np.empty((N, F4), np.float32)
    rr = np.arange(128)[:, None] + 128 * np.arange(G)[None, :]
    mreal = rr < NLOC
    for c in range(NC):
        mo = np.asarray(res.results[c]["mu"]).reshape(128, G, F4)
        lo = np.asarray(res.results[c]["ls"]).reshape(128, G, F4)
        nodes = c * NLOC + cores[c]["order"][rr[mreal]]
        mu[nodes] = mo[mreal]
        ls[nodes] = lo[mreal]
    return mu, ls


# revision 4
# speedup vs baseline: 1.0601x; 1.0601x over previous
"""GCN encoder v2: per-window degree-sorted ELL + partial-table combine.

Same overall scheme as kernel.py, but each of the 4 source windows gets its
own per-core degree sort, so ELL padding is ~1.05x instead of ~2.5x. Each
window pass reduces into a partial table P_r [NPAD, FW] (rank_r order);
partials are then combined with a uniform width-4 gather (2 windows of
2*NPAD rows) and one whole-canvas reduce per chunk.
"""

import numpy as np

N = 100000
NC = 8
NLOC = N // NC
G = 98
NPAD = 128 * G
NFULL = NC * NPAD
WIN = NFULL // 4
F1, F2, F3, F4 = 128, 64, 32, 16
MAX_COLS_PER_GATHER = 64
NCHUNK = 4                      # combine chunks of groups
CH = [25, 25, 24, 24]

_PROFILE = False
_last_exec_ns = None
_TMPDIR = None


def _wrap_idxs(idxs):
    n = len(idxs)
    assert n % 16 == 0
    w = idxs.reshape(n // 16, 16).T.astype(np.int16)
    return np.tile(w, (8, 1))


def _prow(rank):
    return (rank % 128) * G + (rank // 128)


def _preprocess(edge_index):
    src = np.asarray(edge_index[0], dtype=np.int64)
    dst = np.asarray(edge_index[1], dtype=np.int64)
    loop = np.arange(N, dtype=np.int64)
    src = np.concatenate([src, loop])
    dst = np.concatenate([dst, loop])

    deg = np.bincount(dst, minlength=N).astype(np.float64)
    dinv = np.where(deg > 0, 1.0 / np.sqrt(deg), 0.0).astype(np.float32)

    cores = []
    for c in range(NC):
        lo = c * NLOC
        m = (dst >= lo) & (dst < lo + NLOC)
        s_c = src[m]
        d_c = dst[m] - lo
        degloc = np.bincount(d_c, minlength=NLOC)
        order = np.argsort(-degloc, kind="stable")
        rank_of = np.empty(NLOC, np.int64)
        rank_of[order] = np.arange(NLOC)
        cores.append(dict(s=s_c, d=d_c, order=order, rank_of=rank_of))

    row_of_node = np.empty(N, np.int64)
    for c in range(NC):
        rk = cores[c]["rank_of"]
        row_of_node[c * NLOC:(c + 1) * NLOC] = c * NPAD + _prow(rk)

    for c in range(NC):
        cc = cores[c]
        trow = row_of_node[cc["s"]]
        cc["win"] = trow // WIN
        cc["lidx"] = trow % WIN
        # per-window sorts
        cc["order_r"] = []
        cc["rank_r_of"] = []
        cc["deg_r"] = []
        for r in range(4):
            dr = np.bincount(cc["d"][cc["win"] == r], minlength=NLOC)
            o = np.argsort(-dr, kind="stable")
            ro = np.empty(NLOC, np.int64)
            ro[o] = np.arange(NLOC)
            cc["order_r"].append(o)
            cc["rank_r_of"].append(ro)
            cc["deg_r"].append(dr)

    # per-window per-group widths (cross-core max); sorted desc so
    # W_r[g] = max over cores of deg_r[order_r[128*g]]
    Wr = np.zeros((4, G), np.int32)
    for r in range(4):
        for c in range(NC):
            cc = cores[c]
            top = cc["deg_r"][r][cc["order_r"][r][::128][:G]]
            Wr[r] = np.maximum(Wr[r], top)

    zero_local = _prow(NLOC)    # pad-rank row, zero in every table window

    # pass gather index streams
    idx_pass = []               # [core][r] -> [128, 8*sum(Wr[r])]
    for c in range(NC):
        cc = cores[c]
        per_r = []
        for r in range(4):
            m = cc["win"] == r
            d_r = cc["d"][m]
            li_r = cc["lidx"][m]
            rk = cc["rank_r_of"][r][d_r]
            ordk = np.lexsort((li_r, rk))
            rk_s, li_s = rk[ordk], li_r[ordk]
            start = np.searchsorted(rk_s, np.arange(NLOC))
            end = np.searchsorted(rk_s, np.arange(NLOC) + 1)
            parts = []
            for g in range(G):
                w = int(Wr[r][g])
                if w == 0:
                    continue
                seg = np.full((128, w), zero_local, np.int64)
                for p in range(128):
                    rr = 128 * g + p
                    if rr < NLOC:
                        a, b = start[rr], end[rr]
                        if b > a:
                            seg[p, :b - a] = li_s[a:b]
                parts.append(_wrap_idxs(seg.T.reshape(-1)))
            per_r.append(np.concatenate(parts, axis=1) if parts
                         else np.zeros((128, 16), np.int16))
        idx_pass.append(per_r)

    # combine index streams: window pair A=(P0,P1), B=(P2,P3); final order =
    # total-degree ranks. slot i of pair X selects partial of pass 2X+i.
    idx_comb = []               # [core][pair] -> [128, 8*2*G]
    pad_prow = _prow(NLOC)
    for c in range(NC):
        cc = cores[c]
        pair_streams = []
        for pair in range(2):
            parts = []
            for g in range(G):
                seg = np.empty((128, 2), np.int64)
                for i in range(2):
                    r = 2 * pair + i
                    rowv = np.full(128, pad_prow, np.int64)
                    rr = 128 * g + np.arange(128)
                    real = rr < NLOC
                    nodes = cc["order"][rr[real]]
                    rowv[real] = _prow(cc["rank_r_of"][r][nodes])
                    seg[:, i] = rowv + i * NPAD
                parts.append(_wrap_idxs(seg.T.reshape(-1)))
            pair_streams.append(np.concatenate(parts, axis=1))
        idx_comb.append(pair_streams)

    return dinv, cores, Wr, idx_pass, idx_comb


def _build_program(Wr, pass_len):
    import contextlib
    import concourse.bacc as bacc
    import concourse.mybir as mybir
    import concourse.tile as tile
    from concourse import library_config
    from concourse.masks import make_identity

    dt = mybir.dt
    Alu = mybir.AluOpType
    nc = bacc.Bacc("TRN2", target_bir_lowering=False, debug=False,
                   num_devices=NC)

    xT = nc.dram_tensor("xT", [128, NPAD], dt.float32, kind="ExternalInput")
    dinv_d = nc.dram_tensor("dinv", [128, G], dt.float32, kind="ExternalInput")
    w1_d = nc.dram_tensor("w1", [F1, F2], dt.float32, kind="ExternalInput")
    w2_d = nc.dram_tensor("w2", [F2, F3], dt.float32, kind="ExternalInput")
    wmu_d = nc.dram_tensor("wmu", [F3, F4], dt.float32, kind="ExternalInput")
    wls_d = nc.dram_tensor("wls", [F3, F4], dt.float32, kind="ExternalInput")
    b1_d = nc.dram_tensor("b1t", [128, F2], dt.float32, kind="ExternalInput")
    b2_d = nc.dram_tensor("b2t", [128, F3], dt.float32, kind="ExternalInput")
    bmu_d = nc.dram_tensor("bmut", [128, F4], dt.float32, kind="ExternalInput")
    bls_d = nc.dram_tensor("blst", [128, F4], dt.float32, kind="ExternalInput")
    idxp_d = [nc.dram_tensor(f"idxp{r}", [128, pass_len[r]], dt.int16,
                             kind="ExternalInput") for r in range(4)]
    idxc_d = [nc.dram_tensor(f"idxc{p}", [128, 16 * G], dt.int16,
                             kind="ExternalInput") for p in range(2)]
    mu_out = nc.dram_tensor("mu", [128, G, F4], dt.float32,
                            kind="ExternalOutput")
    ls_out = nc.dram_tensor("ls", [128, G, F4], dt.float32,
                            kind="ExternalOutput")

    FW = F2

    with tile.TileContext(nc) as tc:
        with contextlib.ExitStack() as ctx:
            dram = ctx.enter_context(
                tc.tile_pool(name="dram", bufs=1, space="DRAM"))
            consts = ctx.enter_context(tc.tile_pool(name="consts", bufs=1))
            psum_mm = ctx.enter_context(
                tc.tile_pool(name="psum_mm", bufs=3, space="PSUM"))
            psum_tr = ctx.enter_context(
                tc.tile_pool(name="psum_tr", bufs=2, space="PSUM"))
            tabp = ctx.enter_context(tc.tile_pool(name="tabp", bufs=1))
            aggp = ctx.enter_context(tc.tile_pool(name="aggp", bufs=1))
            smallp = ctx.enter_context(tc.tile_pool(name="smallp", bufs=4))

            nc.gpsimd.load_library(library_config.mlp)

            def cload(name, dram_t, shape):
                t = consts.tile(shape, dt.float32, name=name)
                nc.sync.dma_start(t[:], dram_t[:])
                return t

            dinv_sb = cload("dinv_sb", dinv_d, [128, G])
            w1_sb = cload("w1_sb", w1_d, [F1, F2])
            w2_sb = cload("w2_sb", w2_d, [F2, F3])
            wmu_sb = cload("wmu_sb", wmu_d, [F3, F4])
            wls_sb = cload("wls_sb", wls_d, [F3, F4])
            b1_sb = cload("b1_sb", b1_d, [128, F2])
            b2_sb = cload("b2_sb", b2_d, [128, F3])
            bmu_sb = cload("bmu_sb", bmu_d, [128, F4])
            bls_sb = cload("bls_sb", bls_d, [128, F4])
            ident = consts.tile([128, 128], dt.float32, name="ident")
            make_identity(nc, ident[:])

            def store_table(tab_sb, name):
                loc = dram.tile([NPAD, FW], dt.float32, name=name)
                nc.sync.dma_start(
                    loc[:].rearrange("(p g) f -> p g f", p=128), tab_sb[:])
                full = dram.tile([NFULL, FW], dt.float32,
                                 addr_space="Shared", name=name + "_full")
                nc.gpsimd.collective_compute(
                    "AllGather", Alu.bypass,
                    replica_groups=[list(range(NC))],
                    ins=[loc.opt()], outs=[full.opt()],
                )
                return full

            # ---------- Layer 1 matmul ----------
            with tc.tile_pool(name="xTp", bufs=1) as xp:
                xT_sb = xp.tile([128, NPAD], dt.float32, name="xT_sb")
                nc.sync.dma_start(xT_sb[:], xT[:])
                tab_sb = tabp.tile([128, G, FW], dt.float32, tag="tab",
                                   name="tab1_sb")
                for g in range(G):
                    ps = psum_mm.tile([128, FW], dt.float32, space="PSUM",
                                      tag="mm", name=f"mm1_{g}")
                    nc.tensor.matmul(out=ps[:],
                                     lhsT=xT_sb[:, 128 * g:128 * (g + 1)],
                                     rhs=w1_sb[:], start=True, stop=True)
                    nc.vector.tensor_scalar_mul(
                        tab_sb[:, g, :], ps[:], dinv_sb[:, g:g + 1])
                tab1_full = store_table(tab_sb, "tab1")

            idxp = ctx.enter_context(tc.tile_pool(name="idxp", bufs=1))
            msgp = ctx.enter_context(tc.tile_pool(name="msgp", bufs=2))
            combp = ctx.enter_context(tc.tile_pool(name="combp", bufs=2))
            idx_sb = {}
            for r in range(4):
                t = idxp.tile([128, pass_len[r]], dt.int16, tag=f"idx{r}",
                              name=f"idxt{r}")
                nc.sync.dma_start(t[:], idxp_d[r][:])
                idx_sb[r] = t
            idxc_sb = {}
            for p in range(2):
                t = idxp.tile([128, 16 * G], dt.int16, tag=f"idxc{p}",
                              name=f"idxct{p}")
                nc.sync.dma_start(t[:], idxc_d[p][:])
                idxc_sb[p] = t

            def aggregate(tab_full, out_cb, phase):
                # 4 window passes into partial tables
                pairs = []
                for pair in range(2):
                    pab = dram.tile([2 * NPAD, FW], dt.float32,
                                    name=f"pab_{phase}_{pair}")
                    pairs.append(pab)
                for r in range(4):
                    P_sb = tabp.tile([128, G, FW], dt.float32, tag="psb",
                                     name=f"psb_{phase}_{r}")
                    nc.vector.memset(P_sb[:], 0.0)
                    off = 0
                    for g in range(G):
                        w = int(Wr[r][g])
                        if w == 0:
                            continue
                        mt = msgp.tile([128, w, FW], dt.float32, tag="msg",
                                       name=f"m_{phase}_{r}_{g}")
                        c0 = 0
                        while c0 < w:
                            cw = min(w - c0, MAX_COLS_PER_GATHER)
                            nc.gpsimd.dma_gather(
                                mt[:, c0:c0 + cw, :],
                                tab_full[r * WIN:(r + 1) * WIN, :],
                                idx_sb[r][:, off + 8 * c0:off + 8 * (c0 + cw)],
                                128 * cw, 128 * cw, FW,
                                single_packet=False,
                            )
                            c0 += cw
                        off += 8 * w
                        nc.vector.tensor_reduce(
                            P_sb[:, g, :], mt[:].rearrange("p w f -> p f w"),
                            axis=mybir.AxisListType.X, op=Alu.add)
                    nc.sync.dma_start(
                        pairs[r // 2][(r % 2) * NPAD:(r % 2 + 1) * NPAD, :]
                        .rearrange("(p g) f -> p g f", p=128),
                        P_sb[:])
                # combine: width-4 gather over the two pair tables
                gl0 = 0
                for ci in range(NCHUNK):
                    ng = CH[ci]
                    outs = []
                    for pair in range(2):
                        mt = combp.tile([128, 2 * ng, FW], dt.float32,
                                        tag="cmb",
                                        name=f"cm_{phase}_{ci}_{pair}")
                        nc.gpsimd.dma_gather(
                            mt[:], pairs[pair][:, :],
                            idxc_sb[pair][:, 16 * gl0:16 * (gl0 + ng)],
                            128 * 2 * ng, 128 * 2 * ng, FW,
                            single_packet=False,
                        )
                        red = combp.tile([128, ng, FW], dt.float32,
                                         tag="crd",
                                         name=f"cr_{phase}_{ci}_{pair}")
                        nc.vector.tensor_reduce(
                            red[:],
                            mt[:].rearrange("p (g two) f -> p g f two",
                                            two=2),
                            axis=mybir.AxisListType.X, op=Alu.add)
                        outs.append(red)
                    comb = combp.tile([128, ng, FW], dt.float32, tag="cfin",
                                      name=f"cf_{phase}_{ci}")
                    nc.vector.tensor_tensor(comb[:], outs[0][:], outs[1][:],
                                            op=Alu.add)
                    for gi in range(ng):
                        out_cb(gl0 + gi, comb[:, gi, :])
                    gl0 += ng

            # ---------- Layer 1 aggregate -> x1 ----------
            x1_sb = aggp.tile([128, G, F2], dt.float32, tag="x1",
                              name="x1_sb")

            def l1_post(g, red):
                nc.vector.tensor_scalar_mul(red[:], red[:],
                                            dinv_sb[:, g:g + 1])
                nc.vector.tensor_tensor(red[:], red[:], b1_sb[:], op=Alu.add)
                nc.vector.tensor_scalar(x1_sb[:, g, :], red[:], 0.0, None,
                                        Alu.max)

            aggregate(tab1_full, l1_post, "l1")

            # ---------- Layer 2 ----------
            tab_sb2 = tabp.tile([128, G, FW], dt.float32, tag="tab",
                                name="tab2_sb")
            nc.vector.memset(tab_sb2[:], 0.0)
            for g in range(G):
                pt = psum_tr.tile([F2, 128], dt.float32, space="PSUM",
                                  tag="tr", name=f"tr2_{g}")
                nc.tensor.transpose(pt[:], x1_sb[:, g, :], ident[:])
                x1t = smallp.tile([F2, 128], dt.float32, tag="x1t",
                                  name=f"x1t_{g}")
                nc.vector.tensor_copy(x1t[:], pt[:])
                ps = psum_mm.tile([128, FW], dt.float32, space="PSUM",
                                  tag="mm", name=f"mm2_{g}")
                nc.tensor.matmul(out=ps[:, 0:F3], lhsT=x1t[:], rhs=w2_sb[:],
                                 start=True, stop=True)
                nc.vector.tensor_scalar_mul(
                    tab_sb2[:, g, 0:F3], ps[:, 0:F3], dinv_sb[:, g:g + 1])
            tab2_full = store_table(tab_sb2, "tab2")

            x2_sb = aggp.tile([128, G, F3], dt.float32, tag="x2",
                              name="x2_sb")

            def l2_post(g, red):
                nc.vector.tensor_scalar_mul(red[:, 0:F3], red[:, 0:F3],
                                            dinv_sb[:, g:g + 1])
                nc.vector.tensor_tensor(red[:, 0:F3], red[:, 0:F3], b2_sb[:],
                                        op=Alu.add)
                nc.vector.tensor_scalar(x2_sb[:, g, :], red[:, 0:F3], 0.0,
                                        None, Alu.max)

            aggregate(tab2_full, l2_post, "l2")

            # ---------- Layer 3 ----------
            tab_sb3 = tabp.tile([128, G, FW], dt.float32, tag="tab",
                                name="tab3_sb")
            nc.vector.memset(tab_sb3[:], 0.0)
            for g in range(G):
                nc.vector.tensor_scalar_mul(
                    tab_sb3[:, g, 0:F3], x2_sb[:, g, :], dinv_sb[:, g:g + 1])
            tab3_full = store_table(tab_sb3, "tab3")

            mu_sb = aggp.tile([128, G, F4], dt.float32, tag="mu",
                              name="mu_sb")
            ls_sb = aggp.tile([128, G, F4], dt.float32, tag="lsb",
                              name="ls_sb")

            def l3_post(g, red):
                nc.vector.tensor_scalar_mul(red[:, 0:F3], red[:, 0:F3],
                                            dinv_sb[:, g:g + 1])
                pt = psum_tr.tile([F3, 128], dt.float32, space="PSUM",
                                  tag="tr", name=f"tr3_{g}")
                nc.tensor.transpose(pt[:], red[:, 0:F3], ident[:])
                zt = smallp.tile([F3, 128], dt.float32, tag="x1t",
                                 name=f"zt_{g}")
                nc.vector.tensor_copy(zt[:], pt[:])
                pmu = psum_mm.tile([128, FW], dt.float32, space="PSUM",
                                   tag="mm", name=f"pmu_{g}")
                nc.tensor.matmul(out=pmu[:, 0:F4], lhsT=zt[:], rhs=wmu_sb[:],
                                 start=True, stop=True)
                nc.vector.tensor_tensor(mu_sb[:, g, :], pmu[:, 0:F4],
                                        bmu_sb[:], op=Alu.add)
                pls = psum_mm.tile([128, FW], dt.float32, space="PSUM",
                                   tag="mm", name=f"pls_{g}")
                nc.tensor.matmul(out=pls[:, 0:F4], lhsT=zt[:], rhs=wls_sb[:],
                                 start=True, stop=True)
                nc.vector.tensor_tensor(ls_sb[:, g, :], pls[:, 0:F4],
                                        bls_sb[:], op=Alu.add)

            aggregate(tab3_full, l3_post, "l3")

            nc.sync.dma_start(mu_out[:], mu_sb[:])
            nc.sync.dma_start(ls_out[:], ls_sb[:])

    nc.compile()
    return nc


def kernel(x, edge_index, W1, b1, W2, b2, Wmu, bmu, Wls, bls):
    global _last_exec_ns
    x = np.asarray(x, np.float32)
    dinv, cores, Wr, idx_pass, idx_comb = _preprocess(edge_index)
    pass_len = [idx_pass[0][r].shape[1] for r in range(4)]

    nc = _build_program(Wr, pass_len)

    def btile(b):
        return np.tile(np.asarray(b, np.float32)[None, :], (128, 1))

    in_maps = []
    for c in range(NC):
        cc = cores[c]
        xT = np.zeros((128, NPAD), np.float32)
        xT[:, cc["rank_of"]] = x[c * NLOC:(c + 1) * NLOC].T

        dv = np.zeros((128, G), np.float32)
        rr = np.arange(128)[:, None] + 128 * np.arange(G)[None, :]
        mreal = rr < NLOC
        dv[mreal] = dinv[c * NLOC + cc["order"][rr[mreal]]]

        im = dict(xT=xT, dinv=dv, w1=np.asarray(W1, np.float32),
                  w2=np.asarray(W2, np.float32),
                  wmu=np.asarray(Wmu, np.float32),
                  wls=np.asarray(Wls, np.float32),
                  b1t=btile(b1), b2t=btile(b2), bmut=btile(bmu),
                  blst=btile(bls))
        for r in range(4):
            im[f"idxp{r}"] = idx_pass[c][r]
        for p in range(2):
            im[f"idxc{p}"] = idx_comb[c][p]
        in_maps.append(im)

    from concourse.bass_utils import run_bass_kernel_spmd
    res = run_bass_kernel_spmd(nc, in_maps, core_ids=list(range(NC)),
                               trace=_PROFILE, tmpdir=_TMPDIR)
    _last_exec_ns = res.exec_time_ns

    mu = np.empty((N, F4), np.float32)
    ls = np.empty((N, F4), np.float32)
    rr = np.arange(128)[:, None] + 128 * np.arange(G)[None, :]
    mreal = rr < NLOC
    for c in range(NC):
        mo = np.asarray(res.results[c]["mu"]).reshape(128, G, F4)
        lo = np.asarray(res.results[c]["ls"]).reshape(128, G, F4)
        nodes = c * NLOC + cores[c]["order"][rr[mreal]]
        mu[nodes] = mo[mreal]
        ls[nodes] = lo[mreal]
    return mu, ls


# revision 5
# speedup vs baseline: 1.0642x; 1.0038x over previous
"""GCN encoder v2: per-window degree-sorted ELL + partial-table combine.

Same overall scheme as kernel.py, but each of the 4 source windows gets its
own per-core degree sort, so ELL padding is ~1.05x instead of ~2.5x. Each
window pass reduces into a partial table P_r [NPAD, FW] (rank_r order);
partials are then combined with a uniform width-4 gather (2 windows of
2*NPAD rows) and one whole-canvas reduce per chunk.
"""

import numpy as np

N = 100000
NC = 8
NLOC = N // NC
G = 98
NPAD = 128 * G
NFULL = NC * NPAD
WIN = NFULL // 4
F1, F2, F3, F4 = 128, 64, 32, 16
MAX_COLS_PER_GATHER = 64
CHUNK_COLS = 32
NCHUNK = 7                      # combine chunks of groups
CH = [14] * 7

_PROFILE = False
_last_exec_ns = None
_TMPDIR = None


def _wrap_idxs(idxs):
    n = len(idxs)
    assert n % 16 == 0
    w = idxs.reshape(n // 16, 16).T.astype(np.int16)
    return np.tile(w, (8, 1))


def _prow(rank):
    return (rank % 128) * G + (rank // 128)


def _preprocess(edge_index):
    src = np.asarray(edge_index[0], dtype=np.int64)
    dst = np.asarray(edge_index[1], dtype=np.int64)
    loop = np.arange(N, dtype=np.int64)
    src = np.concatenate([src, loop])
    dst = np.concatenate([dst, loop])

    deg = np.bincount(dst, minlength=N).astype(np.float64)
    dinv = np.where(deg > 0, 1.0 / np.sqrt(deg), 0.0).astype(np.float32)

    cores = []
    for c in range(NC):
        lo = c * NLOC
        m = (dst >= lo) & (dst < lo + NLOC)
        s_c = src[m]
        d_c = dst[m] - lo
        degloc = np.bincount(d_c, minlength=NLOC)
        order = np.argsort(-degloc, kind="stable")
        rank_of = np.empty(NLOC, np.int64)
        rank_of[order] = np.arange(NLOC)
        cores.append(dict(s=s_c, d=d_c, order=order, rank_of=rank_of))

    row_of_node = np.empty(N, np.int64)
    for c in range(NC):
        rk = cores[c]["rank_of"]
        row_of_node[c * NLOC:(c + 1) * NLOC] = c * NPAD + _prow(rk)

    for c in range(NC):
        cc = cores[c]
        trow = row_of_node[cc["s"]]
        cc["win"] = trow // WIN
        cc["lidx"] = trow % WIN
        # per-window sorts
        cc["order_r"] = []
        cc["rank_r_of"] = []
        cc["deg_r"] = []
        for r in range(4):
            dr = np.bincount(cc["d"][cc["win"] == r], minlength=NLOC)
            o = np.argsort(-dr, kind="stable")
            ro = np.empty(NLOC, np.int64)
            ro[o] = np.arange(NLOC)
            cc["order_r"].append(o)
            cc["rank_r_of"].append(ro)
            cc["deg_r"].append(dr)

    # per-window per-group widths (cross-core max); sorted desc so
    # W_r[g] = max over cores of deg_r[order_r[128*g]]
    Wr = np.zeros((4, G), np.int32)
    for r in range(4):
        for c in range(NC):
            cc = cores[c]
            top = cc["deg_r"][r][cc["order_r"][r][::128][:G]]
            Wr[r] = np.maximum(Wr[r], top)

    zero_local = _prow(NLOC)    # pad-rank row, zero in every table window

    # pass gather index streams
    idx_pass = []               # [core][r] -> [128, 8*sum(Wr[r])]
    for c in range(NC):
        cc = cores[c]
        per_r = []
        for r in range(4):
            m = cc["win"] == r
            d_r = cc["d"][m]
            li_r = cc["lidx"][m]
            rk = cc["rank_r_of"][r][d_r]
            ordk = np.lexsort((li_r, rk))
            rk_s, li_s = rk[ordk], li_r[ordk]
            start = np.searchsorted(rk_s, np.arange(NLOC))
            end = np.searchsorted(rk_s, np.arange(NLOC) + 1)
            parts = []
            for g in range(G):
                w = int(Wr[r][g])
                if w == 0:
                    continue
                seg = np.full((128, w), zero_local, np.int64)
                for p in range(128):
                    rr = 128 * g + p
                    if rr < NLOC:
                        a, b = start[rr], end[rr]
                        if b > a:
                            seg[p, :b - a] = li_s[a:b]
                parts.append(_wrap_idxs(seg.T.reshape(-1)))
            per_r.append(np.concatenate(parts, axis=1) if parts
                         else np.zeros((128, 16), np.int16))
        idx_pass.append(per_r)

    # combine index streams: window pair A=(P0,P1), B=(P2,P3); final order =
    # total-degree ranks. slot i of pair X selects partial of pass 2X+i.
    idx_comb = []               # [core][pair] -> [128, 8*2*G]
    pad_prow = _prow(NLOC)
    for c in range(NC):
        cc = cores[c]
        pair_streams = []
        for pair in range(2):
            parts = []
            for g in range(G):
                seg = np.empty((128, 2), np.int64)
                for i in range(2):
                    r = 2 * pair + i
                    rowv = np.full(128, pad_prow, np.int64)
                    rr = 128 * g + np.arange(128)
                    real = rr < NLOC
                    nodes = cc["order"][rr[real]]
                    rowv[real] = _prow(cc["rank_r_of"][r][nodes])
                    seg[:, i] = rowv + i * NPAD
                parts.append(_wrap_idxs(seg.T.reshape(-1)))
            pair_streams.append(np.concatenate(parts, axis=1))
        idx_comb.append(pair_streams)

    return dinv, cores, Wr, idx_pass, idx_comb


def _build_program(Wr, pass_len):
    import contextlib
    import concourse.bacc as bacc
    import concourse.mybir as mybir
    import concourse.tile as tile
    from concourse import library_config
    from concourse.masks import make_identity

    dt = mybir.dt
    Alu = mybir.AluOpType
    nc = bacc.Bacc("TRN2", target_bir_lowering=False, debug=False,
                   num_devices=NC)

    xT = nc.dram_tensor("xT", [128, NPAD], dt.float32, kind="ExternalInput")
    dinv_d = nc.dram_tensor("dinv", [128, G], dt.float32, kind="ExternalInput")
    w1_d = nc.dram_tensor("w1", [F1, F2], dt.float32, kind="ExternalInput")
    w2_d = nc.dram_tensor("w2", [F2, F3], dt.float32, kind="ExternalInput")
    wmu_d = nc.dram_tensor("wmu", [F3, F4], dt.float32, kind="ExternalInput")
    wls_d = nc.dram_tensor("wls", [F3, F4], dt.float32, kind="ExternalInput")
    b1_d = nc.dram_tensor("b1t", [128, F2], dt.float32, kind="ExternalInput")
    b2_d = nc.dram_tensor("b2t", [128, F3], dt.float32, kind="ExternalInput")
    bmu_d = nc.dram_tensor("bmut", [128, F4], dt.float32, kind="ExternalInput")
    bls_d = nc.dram_tensor("blst", [128, F4], dt.float32, kind="ExternalInput")
    idxp_d = [nc.dram_tensor(f"idxp{r}", [128, pass_len[r]], dt.int16,
                             kind="ExternalInput") for r in range(4)]
    idxc_d = [nc.dram_tensor(f"idxc{p}", [128, 16 * G], dt.int16,
                             kind="ExternalInput") for p in range(2)]
    mu_out = nc.dram_tensor("mu", [128, G, F4], dt.float32,
                            kind="ExternalOutput")
    ls_out = nc.dram_tensor("ls", [128, G, F4], dt.float32,
                            kind="ExternalOutput")

    FW = F2

    with tile.TileContext(nc) as tc:
        with contextlib.ExitStack() as ctx:
            dram = ctx.enter_context(
                tc.tile_pool(name="dram", bufs=1, space="DRAM"))
            consts = ctx.enter_context(tc.tile_pool(name="consts", bufs=1))
            psum_mm = ctx.enter_context(
                tc.tile_pool(name="psum_mm", bufs=3, space="PSUM"))
            psum_tr = ctx.enter_context(
                tc.tile_pool(name="psum_tr", bufs=2, space="PSUM"))
            tabp = ctx.enter_context(tc.tile_pool(name="tabp", bufs=1))
            aggp = ctx.enter_context(tc.tile_pool(name="aggp", bufs=1))
            smallp = ctx.enter_context(tc.tile_pool(name="smallp", bufs=4))

            nc.gpsimd.load_library(library_config.mlp)

            def cload(name, dram_t, shape):
                t = consts.tile(shape, dt.float32, name=name)
                nc.sync.dma_start(t[:], dram_t[:])
                return t

            dinv_sb = cload("dinv_sb", dinv_d, [128, G])
            w1_sb = cload("w1_sb", w1_d, [F1, F2])
            w2_sb = cload("w2_sb", w2_d, [F2, F3])
            wmu_sb = cload("wmu_sb", wmu_d, [F3, F4])
            wls_sb = cload("wls_sb", wls_d, [F3, F4])
            b1_sb = cload("b1_sb", b1_d, [128, F2])
            b2_sb = cload("b2_sb", b2_d, [128, F3])
            bmu_sb = cload("bmu_sb", bmu_d, [128, F4])
            bls_sb = cload("bls_sb", bls_d, [128, F4])
            ident = consts.tile([128, 128], dt.float32, name="ident")
            make_identity(nc, ident[:])

            def store_table(tab_sb, name):
                loc = dram.tile([NPAD, FW], dt.float32, name=name)
                nc.sync.dma_start(
                    loc[:].rearrange("(p g) f -> p g f", p=128), tab_sb[:])
                full = dram.tile([NFULL, FW], dt.float32,
                                 addr_space="Shared", name=name + "_full")
                nc.gpsimd.collective_compute(
                    "AllGather", Alu.bypass,
                    replica_groups=[list(range(NC))],
                    ins=[loc.opt()], outs=[full.opt()],
                )
                return full

            # ---------- Layer 1 matmul ----------
            with tc.tile_pool(name="xTp", bufs=1) as xp:
                xT_sb = xp.tile([128, NPAD], dt.float32, name="xT_sb")
                nc.sync.dma_start(xT_sb[:], xT[:])
                tab_sb = tabp.tile([128, G, FW], dt.float32, tag="tab",
                                   name="tab1_sb")
                for g in range(G):
                    ps = psum_mm.tile([128, FW], dt.float32, space="PSUM",
                                      tag="mm", name=f"mm1_{g}")
                    nc.tensor.matmul(out=ps[:],
                                     lhsT=xT_sb[:, 128 * g:128 * (g + 1)],
                                     rhs=w1_sb[:], start=True, stop=True)
                    nc.vector.tensor_scalar_mul(
                        tab_sb[:, g, :], ps[:], dinv_sb[:, g:g + 1])
                tab1_full = store_table(tab_sb, "tab1")

            idxp = ctx.enter_context(tc.tile_pool(name="idxp", bufs=1))
            msgp = ctx.enter_context(tc.tile_pool(name="msgp", bufs=2))
            combp = ctx.enter_context(tc.tile_pool(name="combp", bufs=2))
            idx_sb = {}
            for r in range(4):
                t = idxp.tile([128, pass_len[r]], dt.int16, tag=f"idx{r}",
                              name=f"idxt{r}")
                nc.sync.dma_start(t[:], idxp_d[r][:])
                idx_sb[r] = t
            idxc_sb = {}
            for p in range(2):
                t = idxp.tile([128, 16 * G], dt.int16, tag=f"idxc{p}",
                              name=f"idxct{p}")
                nc.sync.dma_start(t[:], idxc_d[p][:])
                idxc_sb[p] = t

            def aggregate(tab_full, out_cb, phase):
                # 4 window passes into partial tables
                pairs = []
                for pair in range(2):
                    pab = dram.tile([2 * NPAD, FW], dt.float32,
                                    name=f"pab_{phase}_{pair}")
                    pairs.append(pab)
                for r in range(4):
                    P_sb = tabp.tile([128, G, FW], dt.float32, tag="psb",
                                     name=f"psb_{phase}_{r}")
                    nc.vector.memset(P_sb[:], 0.0)
                    # pack whole groups into <=CHUNK_COLS-column chunks
                    chunks = []
                    cur, cols = [], 0
                    for g in range(G):
                        w = int(Wr[r][g])
                        if w == 0:
                            continue
                        assert w <= CHUNK_COLS, (r, g, w)
                        if cols + w > CHUNK_COLS:
                            chunks.append((cur, cols))
                            cur, cols = [], 0
                        cur.append((g, w, cols))
                        cols += w
                    if cur:
                        chunks.append((cur, cols))
                    off = 0
                    for ci, (members, cols) in enumerate(chunks):
                        mt = msgp.tile([128, cols, FW], dt.float32,
                                       tag="msg",
                                       name=f"m_{phase}_{r}_{ci}")
                        nc.gpsimd.dma_gather(
                            mt[:], tab_full[r * WIN:(r + 1) * WIN, :],
                            idx_sb[r][:, off:off + 8 * cols],
                            128 * cols, 128 * cols, FW,
                            single_packet=False,
                        )
                        off += 8 * cols
                        for (g, w, co) in members:
                            nc.vector.tensor_reduce(
                                P_sb[:, g, :],
                                mt[:, co:co + w, :]
                                .rearrange("p w f -> p f w"),
                                axis=mybir.AxisListType.X, op=Alu.add)
                    nc.sync.dma_start(
                        pairs[r // 2][(r % 2) * NPAD:(r % 2 + 1) * NPAD, :]
                        .rearrange("(p g) f -> p g f", p=128),
                        P_sb[:])
                # combine: width-4 gather over the two pair tables
                gl0 = 0
                for ci in range(NCHUNK):
                    ng = CH[ci]
                    outs = []
                    for pair in range(2):
                        mt = combp.tile([128, 2 * ng, FW], dt.float32,
                                        tag="cmb",
                                        name=f"cm_{phase}_{ci}_{pair}")
                        nc.gpsimd.dma_gather(
                            mt[:], pairs[pair][:, :],
                            idxc_sb[pair][:, 16 * gl0:16 * (gl0 + ng)],
                            128 * 2 * ng, 128 * 2 * ng, FW,
                            single_packet=False,
                        )
                        red = combp.tile([128, ng, FW], dt.float32,
                                         tag="crd",
                                         name=f"cr_{phase}_{ci}_{pair}")
                        nc.vector.tensor_reduce(
                            red[:],
                            mt[:].rearrange("p (g two) f -> p g f two",
                                            two=2),
                            axis=mybir.AxisListType.X, op=Alu.add)
                        outs.append(red)
                    comb = combp.tile([128, ng, FW], dt.float32, tag="cfin",
                                      name=f"cf_{phase}_{ci}")
                    nc.vector.tensor_tensor(comb[:], outs[0][:], outs[1][:],
                                            op=Alu.add)
                    for gi in range(ng):
                        out_cb(gl0 + gi, comb[:, gi, :])
                    gl0 += ng

            # ---------- Layer 1 aggregate -> x1 ----------
            x1_sb = aggp.tile([128, G, F2], dt.float32, tag="x1",
                              name="x1_sb")

            def l1_post(g, red):
                nc.vector.tensor_scalar_mul(red[:], red[:],
                                            dinv_sb[:, g:g + 1])
                nc.vector.tensor_tensor(red[:], red[:], b1_sb[:], op=Alu.add)
                nc.vector.tensor_scalar(x1_sb[:, g, :], red[:], 0.0, None,
                                        Alu.max)

            aggregate(tab1_full, l1_post, "l1")

            # ---------- Layer 2 ----------
            tab_sb2 = tabp.tile([128, G, FW], dt.float32, tag="tab",
                                name="tab2_sb")
            nc.vector.memset(tab_sb2[:], 0.0)
            for g in range(G):
                pt = psum_tr.tile([F2, 128], dt.float32, space="PSUM",
                                  tag="tr", name=f"tr2_{g}")
                nc.tensor.transpose(pt[:], x1_sb[:, g, :], ident[:])
                x1t = smallp.tile([F2, 128], dt.float32, tag="x1t",
                                  name=f"x1t_{g}")
                nc.vector.tensor_copy(x1t[:], pt[:])
                ps = psum_mm.tile([128, FW], dt.float32, space="PSUM",
                                  tag="mm", name=f"mm2_{g}")
                nc.tensor.matmul(out=ps[:, 0:F3], lhsT=x1t[:], rhs=w2_sb[:],
                                 start=True, stop=True)
                nc.vector.tensor_scalar_mul(
                    tab_sb2[:, g, 0:F3], ps[:, 0:F3], dinv_sb[:, g:g + 1])
            tab2_full = store_table(tab_sb2, "tab2")

            x2_sb = aggp.tile([128, G, F3], dt.float32, tag="x2",
                              name="x2_sb")

            def l2_post(g, red):
                nc.vector.tensor_scalar_mul(red[:, 0:F3], red[:, 0:F3],
                                            dinv_sb[:, g:g + 1])
                nc.vector.tensor_tensor(red[:, 0:F3], red[:, 0:F3], b2_sb[:],
                                        op=Alu.add)
                nc.vector.tensor_scalar(x2_sb[:, g, :], red[:, 0:F3], 0.0,
                                        None, Alu.max)

            aggregate(tab2_full, l2_post, "l2")

            # ---------- Layer 3 ----------
            tab_sb3 = tabp.tile([128, G, FW], dt.float32, tag="tab",
                                name="tab3_sb")
            nc.vector.memset(tab_sb3[:], 0.0)
            for g in range(G):
                nc.vector.tensor_scalar_mul(
                    tab_sb3[:, g, 0:F3], x2_sb[:, g, :], dinv_sb[:, g:g + 1])
            tab3_full = store_table(tab_sb3, "tab3")

            mu_sb = aggp.tile([128, G, F4], dt.float32, tag="mu",
                              name="mu_sb")
            ls_sb = aggp.tile([128, G, F4], dt.float32, tag="lsb",
                              name="ls_sb")

            def l3_post(g, red):
                nc.vector.tensor_scalar_mul(red[:, 0:F3], red[:, 0:F3],
                                            dinv_sb[:, g:g + 1])
                pt = psum_tr.tile([F3, 128], dt.float32, space="PSUM",
                                  tag="tr", name=f"tr3_{g}")
                nc.tensor.transpose(pt[:], red[:, 0:F3], ident[:])
                zt = smallp.tile([F3, 128], dt.float32, tag="x1t",
                                 name=f"zt_{g}")
                nc.vector.tensor_copy(zt[:], pt[:])
                pmu = psum_mm.tile([128, FW], dt.float32, space="PSUM",
                                   tag="mm", name=f"pmu_{g}")
                nc.tensor.matmul(out=pmu[:, 0:F4], lhsT=zt[:], rhs=wmu_sb[:],
                                 start=True, stop=True)
                nc.vector.tensor_tensor(mu_sb[:, g, :], pmu[:, 0:F4],
                                        bmu_sb[:], op=Alu.add)
                pls = psum_mm.tile([128, FW], dt.float32, space="PSUM",
                                   tag="mm", name=f"pls_{g}")
                nc.tensor.matmul(out=pls[:, 0:F4], lhsT=zt[:], rhs=wls_sb[:],
                                 start=True, stop=True)
                nc.vector.tensor_tensor(ls_sb[:, g, :], pls[:, 0:F4],
                                        bls_sb[:], op=Alu.add)

            aggregate(tab3_full, l3_post, "l3")

            nc.sync.dma_start(mu_out[:], mu_sb[:])
            nc.sync.dma_start(ls_out[:], ls_sb[:])

    nc.compile()
    return nc


def kernel(x, edge_index, W1, b1, W2, b2, Wmu, bmu, Wls, bls):
    global _last_exec_ns
    x = np.asarray(x, np.float32)
    dinv, cores, Wr, idx_pass, idx_comb = _preprocess(edge_index)
    pass_len = [idx_pass[0][r].shape[1] for r in range(4)]

    nc = _build_program(Wr, pass_len)

    def btile(b):
        return np.tile(np.asarray(b, np.float32)[None, :], (128, 1))

    in_maps = []
    for c in range(NC):
        cc = cores[c]
        xT = np.zeros((128, NPAD), np.float32)
        xT[:, cc["rank_of"]] = x[c * NLOC:(c + 1) * NLOC].T

        dv = np.zeros((128, G), np.float32)
        rr = np.arange(128)[:, None] + 128 * np.arange(G)[None, :]
        mreal = rr < NLOC
        dv[mreal] = dinv[c * NLOC + cc["order"][rr[mreal]]]

        im = dict(xT=xT, dinv=dv, w1=np.asarray(W1, np.float32),
                  w2=np.asarray(W2, np.float32),
                  wmu=np.asarray(Wmu, np.float32),
                  wls=np.asarray(Wls, np.float32),
                  b1t=btile(b1), b2t=btile(b2), bmut=btile(bmu),
                  blst=btile(bls))
        for r in range(4):
            im[f"idxp{r}"] = idx_pass[c][r]
        for p in range(2):
            im[f"idxc{p}"] = idx_comb[c][p]
        in_maps.append(im)

    from concourse.bass_utils import run_bass_kernel_spmd
    res = run_bass_kernel_spmd(nc, in_maps, core_ids=list(range(NC)),
                               trace=_PROFILE, tmpdir=_TMPDIR)
    _last_exec_ns = res.exec_time_ns

    mu = np.empty((N, F4), np.float32)
    ls = np.empty((N, F4), np.float32)
    rr = np.arange(128)[:, None] + 128 * np.arange(G)[None, :]
    mreal = rr < NLOC
    for c in range(NC):
        mo = np.asarray(res.results[c]["mu"]).reshape(128, G, F4)
        lo = np.asarray(res.results[c]["ls"]).reshape(128, G, F4)
        nodes = c * NLOC + cores[c]["order"][rr[mreal]]
        mu[nodes] = mo[mreal]
        ls[nodes] = lo[mreal]
    return mu, ls


# revision 6
# speedup vs baseline: 1.0695x; 1.0050x over previous
"""GCN encoder v2: per-window degree-sorted ELL + partial-table combine.

Same overall scheme as kernel.py, but each of the 4 source windows gets its
own per-core degree sort, so ELL padding is ~1.05x instead of ~2.5x. Each
window pass reduces into a partial table P_r [NPAD, FW] (rank_r order);
partials are then combined with a uniform width-4 gather (2 windows of
2*NPAD rows) and one whole-canvas reduce per chunk.
"""

import numpy as np

N = 100000
NC = 8
NLOC = N // NC
G = 98
NPAD = 128 * G
NFULL = NC * NPAD
WIN = NFULL // 4
F1, F2, F3, F4 = 128, 64, 32, 16
MAX_COLS_PER_GATHER = 64
CHUNK_COLS = 32
NCHUNK = 7                      # combine chunks of groups
CH = [14] * 7

_PROFILE = False
_last_exec_ns = None
_TMPDIR = None


def _wrap_idxs(idxs):
    n = len(idxs)
    assert n % 16 == 0
    w = idxs.reshape(n // 16, 16).T.astype(np.int16)
    return np.tile(w, (8, 1))


def _prow(rank):
    return (rank % 128) * G + (rank // 128)


def _preprocess(edge_index):
    src = np.asarray(edge_index[0], dtype=np.int64)
    dst = np.asarray(edge_index[1], dtype=np.int64)
    loop = np.arange(N, dtype=np.int64)
    src = np.concatenate([src, loop])
    dst = np.concatenate([dst, loop])

    deg = np.bincount(dst, minlength=N).astype(np.float64)
    dinv = np.where(deg > 0, 1.0 / np.sqrt(deg), 0.0).astype(np.float32)

    cores = []
    for c in range(NC):
        lo = c * NLOC
        m = (dst >= lo) & (dst < lo + NLOC)
        s_c = src[m]
        d_c = dst[m] - lo
        degloc = np.bincount(d_c, minlength=NLOC)
        order = np.argsort(-degloc, kind="stable")
        rank_of = np.empty(NLOC, np.int64)
        rank_of[order] = np.arange(NLOC)
        cores.append(dict(s=s_c, d=d_c, order=order, rank_of=rank_of))

    row_of_node = np.empty(N, np.int64)
    for c in range(NC):
        rk = cores[c]["rank_of"]
        row_of_node[c * NLOC:(c + 1) * NLOC] = c * NPAD + _prow(rk)

    for c in range(NC):
        cc = cores[c]
        trow = row_of_node[cc["s"]]
        cc["win"] = trow // WIN
        cc["lidx"] = trow % WIN
        # per-window sorts
        cc["order_r"] = []
        cc["rank_r_of"] = []
        cc["deg_r"] = []
        for r in range(4):
            dr = np.bincount(cc["d"][cc["win"] == r], minlength=NLOC)
            o = np.argsort(-dr, kind="stable")
            ro = np.empty(NLOC, np.int64)
            ro[o] = np.arange(NLOC)
            cc["order_r"].append(o)
            cc["rank_r_of"].append(ro)
            cc["deg_r"].append(dr)

    # per-window per-group widths (cross-core max); sorted desc so
    # W_r[g] = max over cores of deg_r[order_r[128*g]]
    Wr = np.zeros((4, G), np.int32)
    for r in range(4):
        for c in range(NC):
            cc = cores[c]
            top = cc["deg_r"][r][cc["order_r"][r][::128][:G]]
            Wr[r] = np.maximum(Wr[r], top)

    zero_local = _prow(NLOC)    # pad-rank row, zero in every table window

    # pass gather index streams
    idx_pass = []               # [core][r] -> [128, 8*sum(Wr[r])]
    for c in range(NC):
        cc = cores[c]
        per_r = []
        for r in range(4):
            m = cc["win"] == r
            d_r = cc["d"][m]
            li_r = cc["lidx"][m]
            rk = cc["rank_r_of"][r][d_r]
            ordk = np.lexsort((li_r, rk))
            rk_s, li_s = rk[ordk], li_r[ordk]
            start = np.searchsorted(rk_s, np.arange(NLOC))
            end = np.searchsorted(rk_s, np.arange(NLOC) + 1)
            parts = []
            for g in range(G):
                w = int(Wr[r][g])
                if w == 0:
                    continue
                seg = np.full((128, w), zero_local, np.int64)
                for p in range(128):
                    rr = 128 * g + p
                    if rr < NLOC:
                        a, b = start[rr], end[rr]
                        if b > a:
                            seg[p, :b - a] = li_s[a:b]
                parts.append(_wrap_idxs(seg.T.reshape(-1)))
            per_r.append(np.concatenate(parts, axis=1) if parts
                         else np.zeros((128, 16), np.int16))
        idx_pass.append(per_r)

    # combine index streams: window pair A=(P0,P1), B=(P2,P3); final order =
    # total-degree ranks. slot i of pair X selects partial of pass 2X+i.
    idx_comb = []               # [core][pair] -> [128, 8*2*G]
    pad_prow = _prow(NLOC)
    for c in range(NC):
        cc = cores[c]
        pair_streams = []
        for pair in range(2):
            parts = []
            for g in range(G):
                seg = np.empty((128, 2), np.int64)
                for i in range(2):
                    r = 2 * pair + i
                    rowv = np.full(128, pad_prow, np.int64)
                    rr = 128 * g + np.arange(128)
                    real = rr < NLOC
                    nodes = cc["order"][rr[real]]
                    rowv[real] = _prow(cc["rank_r_of"][r][nodes])
                    seg[:, i] = rowv + i * NPAD
                parts.append(_wrap_idxs(seg.T.reshape(-1)))
            pair_streams.append(np.concatenate(parts, axis=1))
        idx_comb.append(pair_streams)

    return dinv, cores, Wr, idx_pass, idx_comb


def _build_program(Wr, pass_len):
    import contextlib
    import concourse.bacc as bacc
    import concourse.mybir as mybir
    import concourse.tile as tile
    from concourse import library_config
    from concourse.masks import make_identity

    dt = mybir.dt
    Alu = mybir.AluOpType
    nc = bacc.Bacc("TRN2", target_bir_lowering=False, debug=False,
                   num_devices=NC)

    xT = nc.dram_tensor("xT", [128, NPAD], dt.float32, kind="ExternalInput")
    dinv_d = nc.dram_tensor("dinv", [128, G], dt.float32, kind="ExternalInput")
    w1_d = nc.dram_tensor("w1", [F1, F2], dt.float32, kind="ExternalInput")
    w2_d = nc.dram_tensor("w2", [F2, F3], dt.float32, kind="ExternalInput")
    wmu_d = nc.dram_tensor("wmu", [F3, F4], dt.float32, kind="ExternalInput")
    wls_d = nc.dram_tensor("wls", [F3, F4], dt.float32, kind="ExternalInput")
    b1_d = nc.dram_tensor("b1t", [128, F2], dt.float32, kind="ExternalInput")
    b2_d = nc.dram_tensor("b2t", [128, F3], dt.float32, kind="ExternalInput")
    bmu_d = nc.dram_tensor("bmut", [128, F4], dt.float32, kind="ExternalInput")
    bls_d = nc.dram_tensor("blst", [128, F4], dt.float32, kind="ExternalInput")
    idxp_d = [nc.dram_tensor(f"idxp{r}", [128, pass_len[r]], dt.int16,
                             kind="ExternalInput") for r in range(4)]
    idxc_d = [nc.dram_tensor(f"idxc{p}", [128, 16 * G], dt.int16,
                             kind="ExternalInput") for p in range(2)]
    mu_out = nc.dram_tensor("mu", [128, G, F4], dt.float32,
                            kind="ExternalOutput")
    ls_out = nc.dram_tensor("ls", [128, G, F4], dt.float32,
                            kind="ExternalOutput")

    FW = F2

    with tile.TileContext(nc) as tc:
        with contextlib.ExitStack() as ctx:
            dram = ctx.enter_context(
                tc.tile_pool(name="dram", bufs=1, space="DRAM"))
            consts = ctx.enter_context(tc.tile_pool(name="consts", bufs=1))
            psum_mm = ctx.enter_context(
                tc.tile_pool(name="psum_mm", bufs=3, space="PSUM"))
            psum_tr = ctx.enter_context(
                tc.tile_pool(name="psum_tr", bufs=2, space="PSUM"))
            tabp = ctx.enter_context(tc.tile_pool(name="tabp", bufs=1))
            aggp = ctx.enter_context(tc.tile_pool(name="aggp", bufs=1))
            smallp = ctx.enter_context(tc.tile_pool(name="smallp", bufs=4))

            nc.gpsimd.load_library(library_config.mlp)

            def cload(name, dram_t, shape):
                t = consts.tile(shape, dt.float32, name=name)
                nc.sync.dma_start(t[:], dram_t[:])
                return t

            dinv_sb = cload("dinv_sb", dinv_d, [128, G])
            w1_sb = cload("w1_sb", w1_d, [F1, F2])
            w2_sb = cload("w2_sb", w2_d, [F2, F3])
            wmu_sb = cload("wmu_sb", wmu_d, [F3, F4])
            wls_sb = cload("wls_sb", wls_d, [F3, F4])
            b1_sb = cload("b1_sb", b1_d, [128, F2])
            b2_sb = cload("b2_sb", b2_d, [128, F3])
            bmu_sb = cload("bmu_sb", bmu_d, [128, F4])
            bls_sb = cload("bls_sb", bls_d, [128, F4])
            ident = consts.tile([128, 128], dt.float32, name="ident")
            make_identity(nc, ident[:])

            def store_table(tab_sb, name):
                loc = dram.tile([NPAD, FW], dt.float32, name=name)
                nc.sync.dma_start(
                    loc[:].rearrange("(p g) f -> p g f", p=128), tab_sb[:])
                full = dram.tile([NFULL, FW], dt.float32,
                                 addr_space="Shared", name=name + "_full")
                nc.gpsimd.collective_compute(
                    "AllGather", Alu.bypass,
                    replica_groups=[list(range(NC))],
                    ins=[loc.opt()], outs=[full.opt()],
                )
                return full

            # ---------- Layer 1 matmul ----------
            with tc.tile_pool(name="xTp", bufs=1) as xp:
                xT_sb = xp.tile([128, NPAD], dt.float32, name="xT_sb")
                nc.sync.dma_start(xT_sb[:], xT[:])
                tab_sb = tabp.tile([128, G, FW], dt.float32, tag="tab",
                                   name="tab1_sb")
                for g in range(G):
                    ps = psum_mm.tile([128, FW], dt.float32, space="PSUM",
                                      tag="mm", name=f"mm1_{g}")
                    nc.tensor.matmul(out=ps[:],
                                     lhsT=xT_sb[:, 128 * g:128 * (g + 1)],
                                     rhs=w1_sb[:], start=True, stop=True)
                    nc.vector.tensor_scalar_mul(
                        tab_sb[:, g, :], ps[:], dinv_sb[:, g:g + 1])
                tab1_full = store_table(tab_sb, "tab1")

            idxp = ctx.enter_context(tc.tile_pool(name="idxp", bufs=1))
            msgp = ctx.enter_context(tc.tile_pool(name="msgp", bufs=2))
            combp = ctx.enter_context(tc.tile_pool(name="combp", bufs=2))
            idx_sb = {}
            for r in range(4):
                t = idxp.tile([128, pass_len[r]], dt.int16, tag=f"idx{r}",
                              name=f"idxt{r}")
                nc.sync.dma_start(t[:], idxp_d[r][:])
                idx_sb[r] = t
            idxc_sb = {}
            for p in range(2):
                t = idxp.tile([128, 16 * G], dt.int16, tag=f"idxc{p}",
                              name=f"idxct{p}")
                nc.sync.dma_start(t[:], idxc_d[p][:])
                idxc_sb[p] = t

            def aggregate(tab_full, out_cb, phase):
                # 4 window passes into partial tables
                pairs = []
                for pair in range(2):
                    pab = dram.tile([2 * NPAD, FW], dt.float32,
                                    name=f"pab_{phase}_{pair}")
                    pairs.append(pab)
                for r in range(4):
                    P_sb = tabp.tile([128, G, FW], dt.float32, tag="psb",
                                     name=f"psb_{phase}_{r}")
                    nc.vector.memset(P_sb[:], 0.0)
                    # pack whole groups into <=CHUNK_COLS-column chunks
                    chunks = []
                    cur, cols = [], 0
                    for g in range(G):
                        w = int(Wr[r][g])
                        if w == 0:
                            continue
                        assert w <= CHUNK_COLS, (r, g, w)
                        if cols + w > CHUNK_COLS:
                            chunks.append((cur, cols))
                            cur, cols = [], 0
                        cur.append((g, w, cols))
                        cols += w
                    if cur:
                        chunks.append((cur, cols))
                    off = 0
                    for ci, (members, cols) in enumerate(chunks):
                        mt = msgp.tile([128, cols, FW], dt.float32,
                                       tag="msg",
                                       name=f"m_{phase}_{r}_{ci}")
                        nc.gpsimd.dma_gather(
                            mt[:], tab_full[r * WIN:(r + 1) * WIN, :],
                            idx_sb[r][:, off:off + 8 * cols],
                            128 * cols, 128 * cols, FW,
                            single_packet=False,
                        )
                        off += 8 * cols
                        for (g, w, co) in members:
                            nc.vector.tensor_reduce(
                                P_sb[:, g, :],
                                mt[:, co:co + w, :]
                                .rearrange("p w f -> p f w"),
                                axis=mybir.AxisListType.X, op=Alu.add)
                    nc.sync.dma_start(
                        pairs[r // 2][(r % 2) * NPAD:(r % 2 + 1) * NPAD, :]
                        .rearrange("(p g) f -> p g f", p=128),
                        P_sb[:])
                # combine: width-4 gather over the two pair tables
                gl0 = 0
                for ci in range(NCHUNK):
                    ng = CH[ci]
                    outs = []
                    for pair in range(2):
                        mt = combp.tile([128, 2 * ng, FW], dt.float32,
                                        tag="cmb",
                                        name=f"cm_{phase}_{ci}_{pair}")
                        nc.gpsimd.dma_gather(
                            mt[:], pairs[pair][:, :],
                            idxc_sb[pair][:, 16 * gl0:16 * (gl0 + ng)],
                            128 * 2 * ng, 128 * 2 * ng, FW,
                            single_packet=False,
                        )
                        red = combp.tile([128, ng, FW], dt.float32,
                                         tag="crd",
                                         name=f"cr_{phase}_{ci}_{pair}")
                        nc.vector.tensor_reduce(
                            red[:],
                            mt[:].rearrange("p (g two) f -> p g f two",
                                            two=2),
                            axis=mybir.AxisListType.X, op=Alu.add)
                        outs.append(red)
                    comb = combp.tile([128, ng, FW], dt.float32, tag="cfin",
                                      name=f"cf_{phase}_{ci}")
                    nc.vector.tensor_tensor(comb[:], outs[0][:], outs[1][:],
                                            op=Alu.add)
                    for gi in range(ng):
                        out_cb(gl0 + gi, comb[:, gi, :])
                    gl0 += ng

            # ---------- Layer 1 aggregate -> x1 ----------
            x1_cs = [aggp.tile([128, CH[ci], F2], dt.float32,
                               tag=f"x1_{ci}", name=f"x1_sb{ci}")
                     for ci in range(NCHUNK)]

            def l1_post(g, red):
                nc.vector.tensor_scalar_mul(red[:], red[:],
                                            dinv_sb[:, g:g + 1])
                nc.vector.tensor_tensor(red[:], red[:], b1_sb[:], op=Alu.add)
                nc.vector.tensor_scalar(x1_cs[g // 14][:, g % 14, :], red[:],
                                        0.0, None, Alu.max)

            aggregate(tab1_full, l1_post, "l1")

            # ---------- Layer 2 ----------
            tab_sb2 = tabp.tile([128, G, FW], dt.float32, tag="tab",
                                name="tab2_sb")
            nc.vector.memset(tab_sb2[:], 0.0)
            for g in range(G):
                pt = psum_tr.tile([F2, 128], dt.float32, space="PSUM",
                                  tag="tr", name=f"tr2_{g}")
                nc.tensor.transpose(pt[:], x1_cs[g // 14][:, g % 14, :], ident[:])
                x1t = smallp.tile([F2, 128], dt.float32, tag="x1t",
                                  name=f"x1t_{g}")
                nc.vector.tensor_copy(x1t[:], pt[:])
                ps = psum_mm.tile([128, FW], dt.float32, space="PSUM",
                                  tag="mm", name=f"mm2_{g}")
                nc.tensor.matmul(out=ps[:, 0:F3], lhsT=x1t[:], rhs=w2_sb[:],
                                 start=True, stop=True)
                nc.vector.tensor_scalar_mul(
                    tab_sb2[:, g, 0:F3], ps[:, 0:F3], dinv_sb[:, g:g + 1])
            tab2_full = store_table(tab_sb2, "tab2")

            x2_cs = [aggp.tile([128, CH[ci], F3], dt.float32,
                               tag=f"x2_{ci}", name=f"x2_sb{ci}")
                     for ci in range(NCHUNK)]

            def l2_post(g, red):
                nc.vector.tensor_scalar_mul(red[:, 0:F3], red[:, 0:F3],
                                            dinv_sb[:, g:g + 1])
                nc.vector.tensor_tensor(red[:, 0:F3], red[:, 0:F3], b2_sb[:],
                                        op=Alu.add)
                nc.vector.tensor_scalar(x2_cs[g // 14][:, g % 14, :],
                                        red[:, 0:F3], 0.0, None, Alu.max)

            aggregate(tab2_full, l2_post, "l2")

            # ---------- Layer 3 ----------
            tab_sb3 = tabp.tile([128, G, FW], dt.float32, tag="tab",
                                name="tab3_sb")
            nc.vector.memset(tab_sb3[:], 0.0)
            for g in range(G):
                nc.vector.tensor_scalar_mul(
                    tab_sb3[:, g, 0:F3], x2_cs[g // 14][:, g % 14, :],
                    dinv_sb[:, g:g + 1])
            tab3_full = store_table(tab_sb3, "tab3")

            mu_sb = aggp.tile([128, G, F4], dt.float32, tag="mu",
                              name="mu_sb")
            ls_sb = aggp.tile([128, G, F4], dt.float32, tag="lsb",
                              name="ls_sb")

            def l3_post(g, red):
                nc.vector.tensor_scalar_mul(red[:, 0:F3], red[:, 0:F3],
                                            dinv_sb[:, g:g + 1])
                pt = psum_tr.tile([F3, 128], dt.float32, space="PSUM",
                                  tag="tr", name=f"tr3_{g}")
                nc.tensor.transpose(pt[:], red[:, 0:F3], ident[:])
                zt = smallp.tile([F3, 128], dt.float32, tag="x1t",
                                 name=f"zt_{g}")
                nc.vector.tensor_copy(zt[:], pt[:])
                pmu = psum_mm.tile([128, FW], dt.float32, space="PSUM",
                                   tag="mm", name=f"pmu_{g}")
                nc.tensor.matmul(out=pmu[:, 0:F4], lhsT=zt[:], rhs=wmu_sb[:],
                                 start=True, stop=True)
                nc.vector.tensor_tensor(mu_sb[:, g, :], pmu[:, 0:F4],
                                        bmu_sb[:], op=Alu.add)
                pls = psum_mm.tile([128, FW], dt.float32, space="PSUM",
                                   tag="mm", name=f"pls_{g}")
                nc.tensor.matmul(out=pls[:, 0:F4], lhsT=zt[:], rhs=wls_sb[:],
                                 start=True, stop=True)
                nc.vector.tensor_tensor(ls_sb[:, g, :], pls[:, 0:F4],
                                        bls_sb[:], op=Alu.add)

            aggregate(tab3_full, l3_post, "l3")

            nc.sync.dma_start(mu_out[:], mu_sb[:])
            nc.sync.dma_start(ls_out[:], ls_sb[:])

    nc.compile()
    return nc


def kernel(x, edge_index, W1, b1, W2, b2, Wmu, bmu, Wls, bls):
    global _last_exec_ns
    x = np.asarray(x, np.float32)
    dinv, cores, Wr, idx_pass, idx_comb = _preprocess(edge_index)
    pass_len = [idx_pass[0][r].shape[1] for r in range(4)]

    nc = _build_program(Wr, pass_len)

    def btile(b):
        return np.tile(np.asarray(b, np.float32)[None, :], (128, 1))

    in_maps = []
    for c in range(NC):
        cc = cores[c]
        xT = np.zeros((128, NPAD), np.float32)
        xT[:, cc["rank_of"]] = x[c * NLOC:(c + 1) * NLOC].T

        dv = np.zeros((128, G), np.float32)
        rr = np.arange(128)[:, None] + 128 * np.arange(G)[None, :]
        mreal = rr < NLOC
        dv[mreal] = dinv[c * NLOC + cc["order"][rr[mreal]]]

        im = dict(xT=xT, dinv=dv, w1=np.asarray(W1, np.float32),
                  w2=np.asarray(W2, np.float32),
                  wmu=np.asarray(Wmu, np.float32),
                  wls=np.asarray(Wls, np.float32),
                  b1t=btile(b1), b2t=btile(b2), bmut=btile(bmu),
                  blst=btile(bls))
        for r in range(4):
            im[f"idxp{r}"] = idx_pass[c][r]
        for p in range(2):
            im[f"idxc{p}"] = idx_comb[c][p]
        in_maps.append(im)

    from concourse.bass_utils import run_bass_kernel_spmd
    res = run_bass_kernel_spmd(nc, in_maps, core_ids=list(range(NC)),
                               trace=_PROFILE, tmpdir=_TMPDIR)
    _last_exec_ns = res.exec_time_ns

    mu = np.empty((N, F4), np.float32)
    ls = np.empty((N, F4), np.float32)
    rr = np.arange(128)[:, None] + 128 * np.arange(G)[None, :]
    mreal = rr < NLOC
    for c in range(NC):
        mo = np.asarray(res.results[c]["mu"]).reshape(128, G, F4)
        lo = np.asarray(res.results[c]["ls"]).reshape(128, G, F4)
        nodes = c * NLOC + cores[c]["order"][rr[mreal]]
        mu[nodes] = mo[mreal]
        ls[nodes] = lo[mreal]
    return mu, ls
